# revision 10
# baseline (speedup 1.0000x reference)
"""BEV camera-to-grid scatter kernel v2 for Trainium2 (8 NeuronCores).

Differences from v1:
 - j-groups are 16-row x 8-col image patches; per-patch BEV x-bands from
   interval arithmetic -> one-hot width ~bandw*wy instead of the whole unit
   window -> ~5x fewer one-hot elements.
 - x-major unit windows (lidx = kx_local*wy + ky); band edges double as
   clipping (no pen for x/y, no unit splitting, no scans).
 - feats are fp16 (host-converted); one-hot fp16; matmul fp16xfp16 -> f32 PSUM.
 - batched binning: one compare + one reduce per axis for all units.
 - optional on-device repeat loop (tc.For_i) for timing.
"""
import sys
import numpy as np

sys.path.insert(0, '/opt/trn_rl_repo')

B, N, D, FH, FW, C = 1, 6, 118, 32, 88, 80
IH, IW = 256, 704
NX, NY, NZ = 360, 360, 1
DXS = (0.3, 0.3, 20.0)
COFF = (-54.0, -54.0, -10.0)
NCORES = 8
HHALF = 16
PCOLS = 8
UJ = FW // PCOLS            # 11 patches per unit
UPIX = 128 * UJ             # 1408
NCOEF = 15
BIGPEN = 1.0e6
SIM_MODE = 'full'
SEGY = 16                   # y threshold slots per unit (wy+1 <= 15 observed)


def _frustum_axes():
    ds = np.arange(1.0, 60.0, 0.5, dtype=np.float32)
    xs = np.linspace(0.0, IW - 1, FW, dtype=np.float32)
    ys = np.linspace(0.0, IH - 1, FH, dtype=np.float32)
    return ds, xs, ys


def _compute_coeffs(camera2ego, lidar2ego, camera_intrinsics, img_aug_matrix, lidar_aug_matrix):
    aug = np.asarray(img_aug_matrix, np.float64)
    c2e = np.asarray(camera2ego, np.float64)
    intr = np.asarray(camera_intrinsics, np.float64)
    l2e = np.asarray(lidar2ego, np.float64)
    laug = np.asarray(lidar_aug_matrix, np.float64)
    inv_pr = np.linalg.inv(aug[..., :3, :3])
    post_trans = aug[..., :3, 3]
    A64 = inv_pr
    b64 = -np.einsum('bnij,bnj->bni', inv_pr, post_trans)
    combine = c2e[..., :3, :3] @ np.linalg.inv(intr[..., :3, :3])
    pre = laug[..., :3, :3] @ np.linalg.inv(l2e[..., :3, :3])
    M64 = np.einsum('bij,bnjk->bnik', pre, combine)
    t64 = np.einsum('bij,bnj->bni', pre, c2e[..., :3, 3] - l2e[..., :3, 3][:, None, :]) \
        + laug[..., :3, 3][:, None, :]
    return (A64[0].astype(np.float32), b64[0].astype(np.float32),
            M64[0].astype(np.float32), t64[0].astype(np.float32),
            A64[0], b64[0], M64[0], t64[0])


def _compute_thresholds():
    """Exact f32 thresholds replicating trunc((g - COFF)/dx) binning."""
    out = []
    for ax, nb in ((0, NX), (1, NY), (2, NZ)):
        coff = np.float32(COFF[ax]); dx = np.float32(DXS[ax])

        def q_of(g):
            return np.float32(np.float32(np.float32(g) - coff) / dx)

        def smallest(pred, lo, hi):
            def key(i):
                return np.int64(i) if i >= 0 else np.int64(-2147483648) - np.int64(i)
            def unkey(k):
                return np.int32(k) if k >= 0 else np.int32(-(k + 2147483648))
            kl = key(np.float32(lo).view(np.int32)); kh = key(np.float32(hi).view(np.int32))
            assert not pred(unkey(kl).view(np.float32)) and pred(unkey(kh).view(np.float32))
            while kh - kl > 1:
                km = (kl + kh) // 2
                if pred(unkey(km).view(np.float32)):
                    kh = km
                else:
                    kl = km
            return unkey(kh).view(np.float32)

        lo_p = np.float32(coff - 4 * dx); hi_p = np.float32(coff + (nb + 4) * dx)
        L = np.empty(nb + 1, np.float32)
        L[0] = smallest(lambda g: q_of(g) > np.float32(-1.0), lo_p, hi_p)
        for k in range(1, nb + 1):
            L[k] = smallest(lambda g, k=k: q_of(g) >= np.float32(k), lo_p, hi_p)
        out.append(L)
    return out


class _Iv:
    __slots__ = ('lo', 'hi')
    def __init__(self, lo, hi):
        self.lo = float(min(lo, hi)); self.hi = float(max(lo, hi))
    def __add__(self, o):
        if isinstance(o, _Iv):
            return _Iv(self.lo + o.lo, self.hi + o.hi)
        return _Iv(self.lo + o, self.hi + o)
    def __mul__(self, o):
        if isinstance(o, _Iv):
            c = [self.lo * o.lo, self.lo * o.hi, self.hi * o.lo, self.hi * o.hi]
            return _Iv(min(c), max(c))
        return _Iv(self.lo * o, self.hi * o) if o >= 0 else _Iv(self.hi * o, self.lo * o)
    __rmul__ = __mul__
    def intersect(self, o):
        lo = max(self.lo, o.lo); hi = min(self.hi, o.hi)
        return _Iv(lo, hi) if lo <= hi else None
    def pad(self, e):
        return _Iv(self.lo - e, self.hi + e)


def _plan_units(A, b, M, t, Lx, Ly, Lz):
    """Units (cam, depth, h-half) with per-patch x-bands; band gaps filled."""
    ds, xs, ys = _frustum_axes()
    EPS = 2e-3
    zlo, zhi = float(Lz[0]), float(Lz[1])
    units = []
    for n in range(N):
        An = A[n].astype(np.float64); bn = b[n].astype(np.float64)
        Mn = M[n].astype(np.float64); tn = t[n].astype(np.float64)
        for d in range(D):
            dv = float(ds[d])
            for half in range(FH // HHALF):
                pyI = _Iv(float(ys[half * HHALF]), float(ys[half * HHALF + HHALF - 1]))
                patches = []
                for cix in range(UJ):
                    pxI = _Iv(float(xs[cix * PCOLS]), float(xs[cix * PCOLS + PCOLS - 1]))
                    p0 = [(An[i, 0] * pxI + An[i, 1] * pyI + (An[i, 2] * dv + bn[i])).pad(EPS)
                          for i in range(3)]
                    zI = p0[2]
                    qI = (Mn[2, 0] * p0[0] + Mn[2, 1] * p0[1] + Mn[2, 2]).pad(1e-6)
                    gzI = (zI * qI + tn[2]).pad(EPS)
                    if gzI.intersect(_Iv(zlo - EPS, zhi + EPS)) is None:
                        continue
                    zc = zI
                    if qI.lo > 1e-6 or qI.hi < -1e-6:
                        cands = [(zlo - EPS - tn[2]) / qI.lo, (zlo - EPS - tn[2]) / qI.hi,
                                 (zhi + EPS - tn[2]) / qI.lo, (zhi + EPS - tn[2]) / qI.hi]
                        zc = zI.intersect(_Iv(min(cands), max(cands))) or zI
                    rxI = (Mn[0, 0] * p0[0] + Mn[0, 1] * p0[1] + Mn[0, 2]).pad(1e-6)
                    ryI = (Mn[1, 0] * p0[0] + Mn[1, 1] * p0[1] + Mn[1, 2]).pad(1e-6)
                    gxI = (zc * rxI + tn[0]).pad(EPS)
                    gyI = (zc * ryI + tn[1]).pad(EPS)
                    kxa = max(0, int(np.searchsorted(Lx, np.float32(gxI.lo), 'right')) - 1)
                    kxb = min(NX - 1, int(np.searchsorted(Lx, np.float32(gxI.hi), 'right')) - 1)
                    kya = max(0, int(np.searchsorted(Ly, np.float32(gyI.lo), 'right')) - 1)
                    kyb = min(NY - 1, int(np.searchsorted(Ly, np.float32(gyI.hi), 'right')) - 1)
                    if kxb < kxa or kyb < kya:
                        continue
                    kxa = max(0, kxa - 1); kxb = min(NX - 1, kxb + 1)
                    kya = max(0, kya - 1); kyb = min(NY - 1, kyb + 1)
                    patches.append(dict(c=cix, kxa=kxa, kxb=kxb, kya=kya, kyb=kyb))
                if not patches:
                    continue
                # fill x gaps so the band union is contiguous (stale-PSUM
                # safety for the region-add read); operate in band order
                byx = sorted(patches, key=lambda p: (p['kxa'], p['kxb']))
                maxb = byx[0]['kxb']
                for i in range(1, len(byx)):
                    if byx[i]['kxa'] > maxb + 1:
                        byx[i]['kxa'] = maxb + 1
                    maxb = max(maxb, byx[i]['kxb'])
                patches.sort(key=lambda p: p['c'])
                kx0 = min(p['kxa'] for p in patches); kx1 = max(p['kxb'] for p in patches)
                ky0 = min(p['kya'] for p in patches); ky1 = max(p['kyb'] for p in patches)
                wy = ky1 - ky0 + 1; wx = kx1 - kx0 + 1
                assert wy + 1 <= SEGY, f"wy {wy} exceeds SEGY-1"
                assert wx * (wy + 2) <= 1024, f"unit window too large {wx}x{wy}"
                units.append(dict(n=n, d=d, half=half, patches=patches,
                                  kx0=kx0, ky0=ky0, wx=wx, wy=wy))
    return units


def _build_plan(inputs):
    A, b, M, t, A64, b64, M64, t64 = _compute_coeffs(
        inputs['camera2ego'], inputs['lidar2ego'],
        inputs['camera_intrinsics'], inputs['img_aug_matrix'],
        inputs['lidar_aug_matrix'])
    Lx, Ly, Lz = _compute_thresholds()
    units = _plan_units(A, b, M, t, Lx, Ly, Lz)
    assert units, "no units survived culling"
    SEGB = max(p['kxb'] - p['kxa'] + 2 for u in units for p in u['patches'])
    rx0 = min(u['kx0'] for u in units); rx1 = max(u['kx0'] + u['wx'] for u in units)
    ry0 = min(u['ky0'] for u in units); ry1 = max(u['ky0'] + u['wy'] for u in units)
    Rx, Ry = rx1 - rx0, ry1 - ry0
    rcells = Rx * Ry

    # LPT balance across cores by approx DVE cost (one-hot + add)
    def ucost(u):
        mb = max(p['kxb'] - p['kxa'] + 1 for p in u['patches'])
        return len(u['patches']) * mb * u['wy'] + u['wx'] * u['wy'] + 400
    order = sorted(range(len(units)), key=lambda i: -ucost(units[i]))
    loads = [0.0] * NCORES
    percore = [[] for _ in range(NCORES)]
    for i in order:
        k = min(range(NCORES), key=lambda c: loads[c])
        percore[k].append(i)
        loads[k] += ucost(units[i])
    # sort each core's units by window position, then form scatter groups of
    # consecutive units sharing one PSUM tile + one region add
    MAXGRP = 6
    coregroups = []
    for k in range(NCORES):
        percore[k].sort(key=lambda i: (units[i]['ky0'], units[i]['kx0']))
        groups = []
        cur = []
        def gdims(idxs):
            ky0 = min(units[i]['ky0'] for i in idxs)
            ky1 = max(units[i]['ky0'] + units[i]['wy'] - 1 for i in idxs)
            kx0 = min(units[i]['kx0'] for i in idxs)
            kx1 = max(units[i]['kx0'] + units[i]['wx'] - 1 for i in idxs)
            return kx0, ky0, kx1 - kx0 + 1, ky1 - ky0 + 1
        for i in percore[k]:
            trial = cur + [i]
            kx0, ky0, gwx, gwy = gdims(trial)
            if cur and (len(trial) > MAXGRP or gwy + 1 > SEGY
                        or gwx * (gwy + 2) > 1024):
                groups.append(cur)
                cur = [i]
            else:
                cur = trial
        if cur:
            groups.append(cur)
        coregroups.append(groups)
    smax = max(len(p) for p in percore)
    SJ = smax * UJ

    ds, xs, ys = _frustum_axes()
    p_arr = np.arange(128)
    prow = p_arr // PCOLS            # 0..15 within half
    pcol = p_arr % PCOLS             # 0..7 within patch
    f32 = np.float32

    plan = dict(Lx=Lx, Ly=Ly, Lz=Lz, rx0=rx0, ry0=ry0, Rx=Rx, Ry=Ry, rcells=rcells,
                smax=smax, SEGB=SEGB, cores=[])
    for k in range(NCORES):
        glist = []
        pxt = np.zeros((128, SJ), np.float32)
        pyt = np.zeros((128, SJ), np.float32)
        coef = np.zeros((smax, NCOEF), np.float32)
        thrx = np.full((smax * UJ * SEGB,), 3.0e38, np.float32)
        thry = np.full((smax * SEGY,), 3.0e38, np.float32)
        wyb = np.ones((smax,), np.float32)
        cm1 = np.zeros((smax,), np.float32)
        slot = 0
        for gidx in coregroups[k]:
            gky0 = min(units[i]['ky0'] for i in gidx)
            gky1 = max(units[i]['ky0'] + units[i]['wy'] - 1 for i in gidx)
            gkx0 = min(units[i]['kx0'] for i in gidx)
            gkx1 = max(units[i]['kx0'] + units[i]['wx'] - 1 for i in gidx)
            gwy = gky1 - gky0 + 1
            gwx = gkx1 - gkx0 + 1
            gwyS = gwy + 2   # guard cell below and above (no y aliasing)
            assert gwx * gwyS <= 1024
            members = []
            for i in gidx:
                u = units[i]
                s = slot; slot += 1
                n, d, half = u['n'], u['d'], u['half']
                dv = ds[d]
                for j in range(UJ):
                    pxt[:, s * UJ + j] = xs[j * PCOLS + pcol]
                    pyt[:, s * UJ + j] = ys[half * HHALF + prow]
                An64, bn64 = A64[n], b64[n]
                Mn64, tn64 = M64[n], t64[n]
                c2v = An64[:, 2] * dv + bn64        # [3] f64 affine consts of p0
                cc = [An64[2, 0], An64[2, 1], c2v[2]]
                for kk in range(3):
                    ai = Mn64[kk, 0] * An64[0, 0] + Mn64[kk, 1] * An64[1, 0]
                    bi = Mn64[kk, 0] * An64[0, 1] + Mn64[kk, 1] * An64[1, 1]
                    ci = Mn64[kk, 0] * c2v[0] + Mn64[kk, 1] * c2v[1] + Mn64[kk, 2]
                    cc += [ai, bi, ci]
                cc += [tn64[0], tn64[1], tn64[2]]
                coef[s] = np.array(cc, np.float32)
                wyb[s] = gwyS
                cm1[s] = -gwyS
                thry[s * SEGY: s * SEGY + gwy + 1] = Ly[gky0: gky0 + gwy + 1]
                mb = max(p['kxb'] - p['kxa'] + 1 for p in u['patches'])
                for p in u['patches']:
                    bw = p['kxb'] - p['kxa'] + 1
                    o = (s * UJ + p['c']) * SEGB
                    thrx[o: o + bw + 1] = Lx[p['kxa']: p['kxa'] + bw + 1]
                members.append(dict(slot=s, n=n, d=d, half=half, wjp=mb * gwyS,
                                    patches=[dict(c=p['c'],
                                                  base=(p['kxa'] - gkx0) * gwyS,
                                                  wj=(p['kxb'] - p['kxa'] + 1) * gwyS)
                                             for p in u['patches']]))
            glist.append(dict(members=members, wx=gwx, wy=gwy, wyS=gwyS,
                              rxo=gkx0 - rx0, ryo=gky0 - ry0))
        for s in range(slot, smax):
            coef[s] = 0.0
            coef[s][14] = 1.0e9   # tz -> gz=1e9 -> invalid -> lidx inf
        coef_t = np.broadcast_to(coef.reshape(1, smax * NCOEF), (128, smax * NCOEF)).copy()
        thrx_t = np.broadcast_to(thrx.reshape(1, -1), (128, thrx.size)).copy()
        thry_t = np.broadcast_to(thry.reshape(1, -1), (128, thry.size)).copy()
        wyb_t = np.broadcast_to(wyb.reshape(1, -1), (128, smax)).copy()
        cm1_t = np.broadcast_to(cm1.reshape(1, -1), (128, smax)).copy()
        plan['cores'].append(dict(groups=glist, pxt=pxt, pyt=pyt, coef=coef_t,
                                  thrx=thrx_t, thry=thry_t, wyb=wyb_t, cm1=cm1_t,
                                  real=slot))
    wjpmax = max((m['wjp'] for c in plan['cores'] for g in c['groups']
                  for m in g['members']), default=128)
    plan['wjpmax'] = wjpmax
    iota = np.broadcast_to(np.arange(wjpmax, dtype=np.float16).reshape(1, -1),
                           (128, wjpmax)).copy()
    plan['iota'] = iota
    return plan


def _pack_feats(cam_feats, plan):
    """Per-core fp16 feats [smax, 128, UJ*C]: [slot][lane p][patch j][chan]."""
    smax = plan['smax']
    cf = np.asarray(cam_feats)[0]        # [N,D,FH,FW,C] f32
    p_arr = np.arange(128)
    prow = p_arr // PCOLS
    pcol = p_arr % PCOLS
    outs = []
    for core in plan['cores']:
        f = np.zeros((smax, 128, UJ * C), np.float16)
        for grp in core['groups']:
            for u in grp['members']:
                blk = cf[u['n'], u['d'], u['half'] * HHALF:(u['half'] + 1) * HHALF]
                g = blk[prow[:, None], (np.arange(UJ)[None, :] * PCOLS) + pcol[:, None]]
                f[u['slot']] = g.reshape(128, UJ * C).astype(np.float16)
        outs.append(f)
    return outs


_CACHE = {}


def declare_io(nc, plan):
    import concourse.mybir as mybir
    smax, rcells, SEGB = plan['smax'], plan['rcells'], plan['SEGB']
    SJ = smax * UJ
    f32, f16 = mybir.dt.float32, mybir.dt.float16
    io = {}
    io['feats'] = nc.dram_tensor("feats", [smax, 128, UJ * C], f16, kind="ExternalInput")
    io['pxt'] = nc.dram_tensor("pxt", [128, SJ], f32, kind="ExternalInput")
    io['pyt'] = nc.dram_tensor("pyt", [128, SJ], f32, kind="ExternalInput")
    io['coef'] = nc.dram_tensor("coef", [128, smax * NCOEF], f32, kind="ExternalInput")
    io['thrx'] = nc.dram_tensor("thrx", [128, SJ * SEGB], f32, kind="ExternalInput")
    io['thry'] = nc.dram_tensor("thry", [128, smax * SEGY], f32, kind="ExternalInput")
    io['wyb'] = nc.dram_tensor("wyb", [128, smax], f32, kind="ExternalInput")
    io['cm1'] = nc.dram_tensor("cm1", [128, smax], f32, kind="ExternalInput")
    io['iota'] = nc.dram_tensor("iota", [128, plan['wjpmax']], f16, kind="ExternalInput")
    io['region_out'] = nc.dram_tensor("region_out", [C, rcells], f32, kind="ExternalOutput")
    return io


def make_in_maps(inputs, plan):
    feats = _pack_feats(inputs['cam_feats'], plan)
    in_maps = []
    for k in range(NCORES):
        cpl = plan['cores'][k]
        in_maps.append(dict(feats=feats[k], pxt=cpl['pxt'], pyt=cpl['pyt'],
                            coef=cpl['coef'], thrx=cpl['thrx'], thry=cpl['thry'],
                            wyb=cpl['wyb'], cm1=cpl['cm1'], iota=plan['iota']))
    return in_maps


def _build_bass(plan, loop_n=1, one_core=None, loop_mode='body'):
    """loop_mode: 'body' = For_i wraps compute (collective outside, once);
    'collective' = For_i wraps only the AllReduce; body runs once."""
    import concourse.bacc as bacc
    import concourse.mybir as mybir
    import concourse.tile as tile
    import contextlib

    smax, rcells, SEGB = plan['smax'], plan['rcells'], plan['SEGB']
    SJ = smax * UJ
    WJPM = plan['wjpmax']
    f32, f16 = mybir.dt.float32, mybir.dt.float16
    AL = mybir.AluOpType

    ncores = 1 if one_core is not None else NCORES
    nc = bacc.Bacc(None, target_bir_lowering=False, num_devices=ncores)
    io = declare_io(nc, plan)
    pid = nc.partition_id() if one_core is None else None

    Lz = plan['Lz']
    LZ0, LZ1 = float(Lz[0]), float(Lz[1])
    GRP = 6   # units per feats-DMA group

    with tile.TileContext(nc) as tc:
        with tc.tile_pool(name="tabs", bufs=1) as tp, \
             tc.tile_pool(name="geo", bufs=1) as gp, \
             tc.tile_pool(name="fb", bufs=3) as fp_, \
             tc.tile_pool(name="oh", bufs=2) as op_, \
             tc.tile_pool(name="ps", bufs=3, space="PSUM") as pp, \
             tc.tile_pool(name="dram", bufs=1, space="DRAM") as dp:

            pxt = tp.tile([128, SJ], f32); nc.sync.dma_start(pxt[:], io['pxt'][:])
            pyt = tp.tile([128, SJ], f32); nc.sync.dma_start(pyt[:], io['pyt'][:])
            coef = tp.tile([128, smax * NCOEF], f32); nc.sync.dma_start(coef[:], io['coef'][:])
            thrx = tp.tile([128, SJ * SEGB], f32); nc.sync.dma_start(thrx[:], io['thrx'][:])
            thry = tp.tile([128, smax * SEGY], f32); nc.sync.dma_start(thry[:], io['thry'][:])
            wyb = tp.tile([128, smax], f32); nc.sync.dma_start(wyb[:], io['wyb'][:])
            cm1 = tp.tile([128, smax], f32); nc.sync.dma_start(cm1[:], io['cm1'][:])
            iota = tp.tile([128, WJPM], f16); nc.sync.dma_start(iota[:], io['iota'][:])
            zoh = tp.tile([128, 512], f16)
            nc.vector.memset(zoh[:], 0.0)
            rpart = dp.tile([C, rcells], f32, tag="rpart")
            rsum = dp.tile([C, rcells], f32, tag="rsum")

            body_loop = loop_n if (loop_n > 1 and loop_mode == 'body') else 1
            coll_loop = loop_n if (loop_n > 1 and loop_mode == 'collective') else 1
            loop_cm = tc.For_i(0, body_loop) if body_loop > 1 else contextlib.nullcontext()
            with loop_cm:
                region = gp.tile([C, rcells], f32, tag="region")
                nc.vector.memset(region[:], 0.0)

                def cslice(kidx):
                    ap = coef[:].rearrange("p (s k) -> p s k", k=NCOEF)[:, :, kidx:kidx + 1]
                    return ap.broadcast_to([128, smax, UJ])

                def g3(ap):
                    return ap.rearrange("p (s j) -> p s j", j=UJ)

                # ---- batched geometry (g_i = p0z * inner_i + t_i) ----
                tmpa = gp.tile([128, SJ], f32, tag="tmpa")
                tmpb = gp.tile([128, SJ], f32, tag="tmpb")
                p0z = gp.tile([128, SJ], f32, tag="p0z")
                nc.vector.tensor_tensor(out=g3(tmpa[:]), in0=g3(pxt[:]), in1=cslice(0), op=AL.mult)
                nc.vector.tensor_tensor(out=g3(tmpb[:]), in0=g3(pyt[:]), in1=cslice(1), op=AL.mult)
                nc.vector.tensor_tensor(out=tmpa[:], in0=tmpa[:], in1=tmpb[:], op=AL.add)
                nc.vector.tensor_tensor(out=g3(p0z[:]), in0=g3(tmpa[:]), in1=cslice(2), op=AL.add)
                g = [gp.tile([128, SJ], f32, name=f'g_{i}', tag=f'g_{i}') for i in range(3)]
                tmpc = gp.tile([128, SJ], f32, tag="tmpc")
                tmpd = gp.tile([128, SJ], f32, tag="tmpd")
                # gx chain on DVE; gy and gz chains on gpsimd (parallel)
                for kk, eng, (ta, tb) in ((0, nc.vector, (tmpa, tmpb)),
                                          (1, nc.vector, (tmpa, tmpb)),
                                          (2, nc.vector, (tmpa, tmpb))):
                    base = 3 + 3 * kk
                    eng.tensor_tensor(out=g3(ta[:]), in0=g3(pxt[:]), in1=cslice(base + 0), op=AL.mult)
                    eng.tensor_tensor(out=g3(tb[:]), in0=g3(pyt[:]), in1=cslice(base + 1), op=AL.mult)
                    eng.tensor_tensor(out=ta[:], in0=ta[:], in1=tb[:], op=AL.add)
                    eng.tensor_tensor(out=g3(ta[:]), in0=g3(ta[:]), in1=cslice(base + 2), op=AL.add)
                    eng.tensor_tensor(out=ta[:], in0=ta[:], in1=p0z[:], op=AL.mult)
                    eng.tensor_tensor(out=g3(g[kk][:]), in0=g3(ta[:]), in1=cslice(12 + kk), op=AL.add)
                gx, gy, gz = g

                # ---- batched binning ----
                cxb = gp.tile([128, SJ * SEGB], f16, tag="cxb")
                nc.vector.tensor_tensor(
                    out=cxb[:].rearrange("p (s j w) -> p s j w", j=UJ, w=SEGB),
                    in0=g3(gx[:])[:, :, :, None].broadcast_to([128, smax, UJ, SEGB]),
                    in1=thrx[:].rearrange("p (s j w) -> p s j w", j=UJ, w=SEGB),
                    op=AL.is_ge)
                sumx = gp.tile([128, SJ], f16, tag="sumx")
                with nc.allow_low_precision(reason="0/1 counts <= SEGB, exact in fp16"):
                    nc.vector.tensor_reduce(
                        out=g3(sumx[:]),
                        in_=cxb[:].rearrange("p (s j w) -> p s j w", j=UJ, w=SEGB),
                        axis=mybir.AxisListType.X, op=AL.add)
                cyb = gp.tile([128, SJ * SEGY], f16, tag="cyb")
                nc.vector.tensor_tensor(
                    out=cyb[:].rearrange("p (s j w) -> p s j w", j=UJ, w=SEGY),
                    in0=g3(gy[:])[:, :, :, None].broadcast_to([128, smax, UJ, SEGY]),
                    in1=thry[:].rearrange("p (s w) -> p s w", w=SEGY)[:, :, None, :]
                        .broadcast_to([128, smax, UJ, SEGY]),
                    op=AL.is_ge)
                sumy = gp.tile([128, SJ], f16, tag="sumy")
                with nc.allow_low_precision(reason="0/1 counts <= SEGY, exact in fp16"):
                    nc.vector.tensor_reduce(
                        out=g3(sumy[:]),
                        in_=cyb[:].rearrange("p (s j w) -> p s j w", j=UJ, w=SEGY),
                        axis=mybir.AxisListType.X, op=AL.add)
                # valid-z -> additive penalty, folded with -(wy+1)
                nc.vector.tensor_scalar(out=tmpc[:], in0=gz[:], scalar1=LZ0, scalar2=None, op0=AL.is_ge)
                nc.vector.tensor_scalar(out=tmpd[:], in0=gz[:], scalar1=LZ1, scalar2=None, op0=AL.is_lt)
                nc.vector.tensor_tensor(out=tmpc[:], in0=tmpc[:], in1=tmpd[:], op=AL.mult)
                nc.vector.tensor_scalar(out=tmpc[:], in0=tmpc[:], scalar1=-BIGPEN, scalar2=BIGPEN,
                                        op0=AL.mult, op1=AL.add)
                nc.vector.tensor_tensor(
                    out=g3(tmpc[:]), in0=g3(tmpc[:]),
                    in1=cm1[:][:, :, None].broadcast_to([128, smax, UJ]), op=AL.add)
                # lidx = sumx*wy + sumy + pen
                lidx = gp.tile([128, SJ], f32, tag="lidx")
                nc.vector.tensor_tensor(
                    out=g3(lidx[:]), in0=g3(sumx[:]),
                    in1=wyb[:][:, :, None].broadcast_to([128, smax, UJ]), op=AL.mult)
                nc.vector.tensor_tensor(out=lidx[:], in0=lidx[:], in1=sumy[:], op=AL.add)
                nc.vector.tensor_tensor(out=lidx[:], in0=lidx[:], in1=tmpc[:], op=AL.add)

                region2d = region[:].rearrange("p (x y) -> p x y", y=plan['Ry'])

                # ---- per-core unit sections ----
                for core_id in range(ncores if one_core is None else 1):
                    cid = core_id if one_core is None else one_core
                    cpl = plan['cores'][cid]
                    cm = tc.If(pid == core_id) if one_core is None else contextlib.nullcontext()
                    with cm:
                        for grp in cpl['groups']:
                            mem = grp['members']
                            s0 = mem[0]['slot']
                            nmem = len(mem)
                            wy, wx, wyS = grp['wy'], grp['wx'], grp['wyS']
                            W = wx * wyS
                            if SIM_MODE == 'prologue':
                                continue
                            fb = fp_.tile([128, GRP * UJ * C], f16, tag="fb")
                            nc.sync.dma_start(
                                fb[:, :nmem * UJ * C].rearrange(
                                    "p (s m) -> p s m", m=UJ * C),
                                io['feats'][s0:s0 + nmem]
                                    .rearrange("s p m -> p s m"))
                            oh = op_.tile([128, GRP * UJ * WJPM], f16, tag="oh")
                            for mi, u in enumerate(mem):
                                s = u['slot']
                                wjp = u['wjp']
                                c0 = u['patches'][0]['c']
                                c1 = u['patches'][-1]['c']
                                ncmp = c1 - c0 + 1
                                ohm = oh[:, mi * UJ * WJPM:]
                                for p in u['patches']:
                                    jj = p['c'] - c0
                                    nc.vector.tensor_scalar(
                                        out=ohm[:, jj * wjp: jj * wjp + p['wj']],
                                        in0=iota[:, :p['wj']],
                                        scalar1=lidx[:, s * UJ + p['c']:
                                                     s * UJ + p['c'] + 1],
                                        scalar2=None, op0=AL.is_equal)
                            if SIM_MODE == 'onehot':
                                continue
                            psW = W + (W & 1)
                            ps = pp.tile([C, 1024], mybir.dt.float32, space="PSUM", tag="ups")
                            # zero-fill pass (clears stale PSUM, sets has_written)
                            zsegs = [(0, min(psW, 512))]
                            if psW > 512:
                                zsegs.append((512, psW - 512))
                            for (zo, zw) in zsegs:
                                nc.tensor.matmul(ps[:, zo:zo + zw],
                                                 lhsT=fb[:, 0:C],
                                                 rhs=zoh[:, :zw], start=True, stop=False)
                            for mi, u in enumerate(mem):
                                wjp = u['wjp']
                                c0 = u['patches'][0]['c']
                                npat = len(u['patches'])
                                ohm = oh[:, mi * UJ * WJPM:]
                                for pi, p in enumerate(u['patches']):
                                    j = p['c']
                                    jj = j - c0
                                    last = (mi == nmem - 1) and (pi == npat - 1)
                                    lhs = fb[:, (mi * UJ + j) * C: (mi * UJ + j + 1) * C]
                                    b0, wj = p['base'], p['wj']
                                    if b0 < 512 and b0 + wj > 512:
                                        segs = [(b0, 512 - b0), (512, b0 + wj - 512)]
                                    else:
                                        segs = [(b0, wj)]
                                    for (so, sw) in segs:
                                        nc.tensor.matmul(
                                            ps[:, so:so + sw], lhsT=lhs,
                                            rhs=ohm[:, jj * wjp + (so - b0): jj * wjp + (so - b0) + sw],
                                            start=False, stop=last)
                            if SIM_MODE == 'nomm_add':
                                continue
                            dst = region2d[:, grp['rxo']:grp['rxo'] + wx,
                                           grp['ryo']:grp['ryo'] + wy]
                            nc.vector.tensor_tensor(
                                out=dst, in0=dst,
                                in1=ps[:, :W].rearrange("p (x y) -> p x y", y=wyS)[:, :, 1:1 + wy],
                                op=AL.add)

                # ---- stage partial region to DRAM (inside body loop) ----
                nc.sync.dma_start(rpart[:], region[:])

            # ---- epilogue: allreduce partial regions (outside body loop) ----
            if one_core is None:
                # collectives cannot live inside For_i (per-iteration sem reset
                # desyncs the mesh) -> unroll for the timing variant
                for _ in range(coll_loop):
                    nc.gpsimd.collective_compute(
                        "AllReduce", AL.add,
                        replica_groups=[list(range(NCORES))],
                        ins=[rpart[:]], outs=[rsum[:]])
                nc.sync.dma_start(io['region_out'][:], rsum[:])
            else:
                nc.sync.dma_start(io['region_out'][:], rpart[:])

    nc.compile()
    return nc


def _plan_key(plan):
    return (plan['smax'], plan['SEGB'], plan['rcells'], plan['wjpmax'],
            tuple(tuple((g['wx'], g['wy'], g['wyS'], g['rxo'], g['ryo'],
                         tuple((m['slot'],
                                tuple((p['c'], p['base'], p['wj']) for p in m['patches']))
                               for m in g['members']))
                        for g in c['groups']) for c in plan['cores']))


def kernel(**inputs) -> np.ndarray:
    from concourse.bass_utils import run_bass_kernel_spmd

    plan = _build_plan(inputs)
    key = _plan_key(plan)
    if key not in _CACHE:
        _CACHE.clear()
        _CACHE[key] = _build_bass(plan)
    nc = _CACHE[key]

    in_maps = make_in_maps(inputs, plan)
    r = run_bass_kernel_spmd(nc, in_maps, core_ids=list(range(NCORES)))
    region = r.results[0]['region_out']          # [C, rcells] summed over cores
    out = np.zeros((B, C, NX, NY), np.float32)
    Rx, Ry = plan['Rx'], plan['Ry']
    blk = region.reshape(C, Rx, Ry)
    out[0, :, plan['rx0']:plan['rx0'] + Rx, plan['ry0']:plan['ry0'] + Ry] = blk
    return out


# ---------------- host-side numpy emulation (plan/table validation) ---------

def emulate(inputs, plan):
    """Replicate the device computation in numpy from the packed tables."""
    feats = _pack_feats(inputs['cam_feats'], plan)
    region = np.zeros((C, plan['rcells']), np.float64)
    Ry = plan['Ry']
    for k in range(NCORES):
        cpl = plan['cores'][k]
        smax = plan['smax']
        pxt, pyt, coefb = cpl['pxt'], cpl['pyt'], cpl['coef'][0].reshape(smax, NCOEF)
        SJ = smax * UJ
        f32 = np.float32
        # geometry in f32, same op order as device
        cs = coefb[:, :]  # [smax, 15]
        c_exp = np.repeat(cs[None, :, :], 128, 0)  # [128, smax, 15]
        px = pxt.reshape(128, smax, UJ); py = pyt.reshape(128, smax, UJ)
        p0z = f32(f32(f32(px * c_exp[:, :, None, 0]) + f32(py * c_exp[:, :, None, 1]))
                  + c_exp[:, :, None, 2])
        gg = []
        for kk in range(3):
            base = 3 + 3 * kk
            a = f32(f32(px * c_exp[:, :, None, base]) + f32(py * c_exp[:, :, None, base + 1]))
            a = f32(a + c_exp[:, :, None, base + 2])
            a = f32(a * p0z)
            gg.append(f32(a + c_exp[:, :, None, 12 + kk]))
        gxv, gyv, gzv = gg
        thrxv = cpl['thrx'][0].reshape(smax, UJ, plan['SEGB'])
        thryv = cpl['thry'][0].reshape(smax, SEGY)
        sumx = (gxv[:, :, :, None] >= thrxv[None]).sum(-1).astype(f32)
        sumy = (gyv[:, :, :, None] >= thryv[None, :, None, :]).sum(-1).astype(f32)
        Lz = plan['Lz']
        v = ((gzv >= f32(Lz[0])) & (gzv < f32(Lz[1]))).astype(f32)
        pen = f32(-BIGPEN * v + BIGPEN) + cpl['cm1'][0][None, :, None]
        lidx = f32(sumx * cpl['wyb'][0][None, :, None]) + sumy + pen
        lidxh = lidx.astype(np.float16)
        fcore = feats[k]  # [smax, 128, UJ*C] fp16
        for grp in cpl['groups']:
            wy, wx, wyS = grp['wy'], grp['wx'], grp['wyS']
            W = wx * wyS
            acc = np.zeros((W, C), np.float64)
            for u in grp['members']:
                s = u['slot']
                for p in u['patches']:
                    j = p['c']
                    li = lidxh[:, s, j].astype(np.float64)  # [128] patch-local idx
                    f = fcore[s, :, j * C:(j + 1) * C].astype(np.float64)  # [128, C]
                    for lane in range(128):
                        lv = li[lane]
                        if lv >= 0 and lv < p['wj'] and lv == int(lv):
                            acc[p['base'] + int(lv)] += f[lane]
            r2 = region.reshape(C, plan['Rx'], Ry)
            r2[:, grp['rxo']:grp['rxo'] + wx, grp['ryo']:grp['ryo'] + wy] += \
                acc.reshape(wx, wyS, C)[:, 1:1 + wy].transpose(2, 0, 1)
    out = np.zeros((B, C, NX, NY), np.float32)
    blk = region.reshape(C, plan['Rx'], Ry).astype(np.float32)
    out[0, :, plan['rx0']:plan['rx0'] + plan['Rx'], plan['ry0']:plan['ry0'] + Ry] = blk
    return out
